# revision 54
# baseline (speedup 1.0000x reference)
"""Trainium2 Bass kernel for MixformerAttention (sparse attention) — v4.

Problem shape (hardcoded):
  x [B=64, N=320, C=768], W_qkv [768, 2304], W_proj [768, 768], b_proj [768]
  H=12 heads, Dh=64, template L=64, search=256. DP over batch on 8 cores.

v4 changes vs the 306us v2 baseline (trace-driven; 262.7-264.5us measured
across runs, +-1.5us run-to-run variance):
  * One-time pad memsets moved DVE->GpSimd: the DVE memsets were queued
    between the q/k psum drains and chopped the first pair (PE never
    sustained HAM warm-up; ran at 1.2 GHz until t=30us).
  * PE warm-up: 24 dummy matmuls on resident tiles run during the initial
    weight/x DMA window so HAM is at K=8/8 when real work starts; the PE
    now stays warm for the whole kernel (was: cold until 30us + 8
    mid-kernel re-throttles).
  * 64-row batch tails packed: the v-projection and output-projection
    tails of a pair's two batches run as one 128-row pass (PE matmul cost
    is column-count only, so half-empty row chunks wasted cycles).
    Staging: ctail (DVE copy from xT) for the v tail, ptail (drained from
    the attn^T psum) for the proj tail. Odd batches' chunk-2 scores are
    emitted into psum partitions 64:128 via a full-width kT lhsT whose
    cols 64:128 are the tail keys, lining up with the packed v-tail rows.
  * emit_B alternates its psum tiles between the pg and ps pools (ps is
    idle during B), doubling the drain slack from 2 to 4 chains — this was
    the single biggest win (~35us): chain k+2 no longer stalls on chain
    k's DVE drain whenever a PV-filler normalize sat between them.
  * Prev-batch PV/at pieces interleave into emit_B after every chain; the
    final flush runs all PVs first, weaves the held-back proj pieces of
    the second-to-last batch over the normalize latency, and round-robins
    final psum tiles across both pools (scores are done by then).
  * attn^T stays on the PE (identity matmuls). A DMA XBAR transpose
    variant was tried and reverted: each InstDmaTransposeAnt blocks its
    issuing engine queue ~1.2us, convoying either the SP prefetch/store
    triggers or the Act exp pipeline no matter where it was routed.
"""

import contextlib
import functools
import time

import numpy as np

import concourse.bacc as bacc
import concourse.mybir as mybir
from concourse.bass_utils import run_bass_kernel_spmd
from concourse.masks import make_identity
from concourse.tile import TileContext

F32 = mybir.dt.float32
F16 = mybir.dt.float16

NCORES = 8
B, N, C = 64, 320, 768
H, DH = 12, 64
KS = C // 128  # 6 contraction subtiles
B_CORE = B // NCORES  # 8
PAIR_TOK = 2 * N  # 640
NPAIR = B_CORE // 2  # 4
TOK_CORE = B_CORE * N  # 2560
SLOT = 85  # psum col stride per head in PV output (6 heads in 510 cols)

FC_ORDER = [0, 6, 1, 7, 2, 8, 3, 9, 4, 10, 5, 11]  # q/k feature chunk order


def build_kernel():
    nc = bacc.Bacc("TRN2", target_bir_lowering=False)
    x_t = nc.dram_tensor("xT16", [C, TOK_CORE], F16, kind="ExternalInput")
    wqkv_t = nc.dram_tensor("W_qkv16", [C, 3 * C], F16, kind="ExternalInput")
    wproj_t = nc.dram_tensor("W_proj16", [C, C], F16, kind="ExternalInput")
    bias_t = nc.dram_tensor("b_proj", [C], F32, kind="ExternalInput")
    out_t = nc.dram_tensor("out", [TOK_CORE, C], F16, kind="ExternalOutput")
    x_ap, out_ap = x_t.ap(), out_t.ap()

    with TileContext(nc) as tc:
        with contextlib.ExitStack() as ctx:
            P = {
                "const": ctx.enter_context(tc.tile_pool(name="const", bufs=1)),
                "stagep": ctx.enter_context(tc.tile_pool(name="stagep", bufs=1)),
                "xT": ctx.enter_context(tc.tile_pool(name="xT", bufs=2)),
                "qkfc": ctx.enter_context(tc.tile_pool(name="qkfc", bufs=3)),
                "outst": ctx.enter_context(tc.tile_pool(name="outst", bufs=2)),
                "rcp": ctx.enter_context(tc.tile_pool(name="rcp", bufs=4)),
                "ctail": ctx.enter_context(tc.tile_pool(name="ctail", bufs=2)),
                "ptail": ctx.enter_context(tc.tile_pool(name="ptail", bufs=2)),
                "pg": ctx.enter_context(tc.tile_pool(name="pg", bufs=2, space="PSUM")),
                "ps": ctx.enter_context(tc.tile_pool(name="ps", bufs=2, space="PSUM")),
            }
            const = P["const"]

            # ---- persistent constants ----
            wqkv16 = const.tile([128, KS, 3 * C], F16, tag="wqkv16")
            wproj16 = const.tile([128, KS, C], F16, tag="wproj16")
            bias_bc = const.tile([128, C], F32, tag="bias_bc")
            ident16 = const.tile([128, 128], F16, tag="ident16")
            warm16 = const.tile([128, 512], F16, tag="warm16")
            make_identity(nc, ident16)

            # ---- persistent double-slotted activation tiles ----
            # per-head padded q (slot = pair parity)
            qTp = const.tile([128, 2, H, PAIR_TOK], F16, tag="qTp")
            # packed kT feature chunks (2 heads per chunk)
            kTpk = const.tile([128, 2, KS, PAIR_TOK], F16, tag="kTpk")
            # v natural with ones column (slot = batch parity)
            va = const.tile([128, 2, 3, H, 66], F16, tag="va")
            # exp(scores) for search queries [key, h, q]  (256 = search q)
            es = const.tile([128, 2, 3, H, 256], F16, tag="es")
            # exp(scores) template [key<=64 padded, h, q0:64]
            esm = const.tile([128, 2, H, 64], F16, tag="esm")
            # attention rows (template 64 padded | search 128 | search 128)
            attn = const.tile([128, 2, 3, C], F16, tag="attn")
            # attn^T per batch [C-part, tok] (tail tokens 256:320 live in
            # the per-pair ptail staging tile instead)
            attnT = const.tile([128, 2, KS, 256], F16, tag="attnT")

            def emit_warmup():
                # keep PE busy through the HAM SHORT window during the
                # initial DMA wait so real matmuls start at 2.4 GHz
                nc.gpsimd.memset(warm16[:, :], 1.0)
                for _ in range(24):
                    tgw = P["ps"].tile([128, 1024], F32, tag="ps")
                    nc.tensor.matmul(
                        tgw[:, 0:512], lhsT=ident16[:, 0:128], rhs=warm16[:, :],
                        start=True, stop=True,
                    )

            def emit_weight_load():
                for ks in range(KS):
                    nc.sync.dma_start(
                        wqkv16[:, ks, :], wqkv_t.ap()[ks * 128 : (ks + 1) * 128, :]
                    )
                for ks in range(KS):
                    nc.sync.dma_start(
                        wproj16[:, ks, :], wproj_t.ap()[ks * 128 : (ks + 1) * 128, :]
                    )
                brow = P["stagep"].tile([128, C], F32, tag="stagep")
                nc.sync.dma_start(brow[0:1, 0:C], bias_t.ap().unsqueeze(0))
                nc.gpsimd.partition_broadcast(bias_bc[:, :], brow[0:1, 0:C])

            def emit_pads():
                # one-time pad zeroing, all on gpsimd (DVE memsets stalled
                # the early q/k psum drains in v2)
                nc.gpsimd.memset(qTp[64:128, :, 0:H:2, :], 0.0)  # even heads
                nc.gpsimd.memset(qTp[0:64, :, 1:H:2, :], 0.0)  # odd heads
                # key-chunk-2 padding: even batches' tail scores live in rows
                # 0:64 (pad 64:128), odd batches' in rows 64:128 (pad 0:64),
                # matching the packed v-tail row split
                nc.gpsimd.memset(es[64:128, 0, 2, :, :], 0.0)
                nc.gpsimd.memset(es[0:64, 1, 2, :, :], 0.0)
                nc.gpsimd.memset(esm[64:128, :, :, :], 0.0)  # template keys
                nc.gpsimd.memset(attn[64:128, :, 0, :], 0.0)  # template rows
                # unused half-rows of the packed v-tails (es rows are zero
                # there, but they must hold finite values for the PV matmul)
                nc.gpsimd.memset(va[64:128, 0, 2, :, :], 0.0)
                nc.gpsimd.memset(va[0:64, 1, 2, :, :], 0.0)
                nc.gpsimd.memset(va[:, :, :, :, 64], 1.0)  # ones column
                nc.gpsimd.memset(va[:, :, :, :, 65], 0.0)  # stride pad

            # ================= emission helpers =================

            def emit_xT(p):
                """DMA the pre-transposed x^T slab of pair p into SBUF.

                Both batches' 64-token tails are staged contiguously in a
                ctail tile (DVE copy from xT: a strided HBM DMA would be
                descriptor-rate-bound) so the packed v-tail matmul gets a
                2D lhsT."""
                xT = P["xT"].tile([128, KS, PAIR_TOK], F16, tag="xT")
                src = x_ap[:, p * PAIR_TOK : (p + 1) * PAIR_TOK]
                nc.sync.dma_start(xT[:], src.rearrange("(k p) t -> p k t", p=128))
                ctail = P["ctail"].tile([128, KS, 128], F16, tag="ctail")
                nc.vector.tensor_copy(ctail[:, :, 0:64], xT[:, :, 256:320])
                nc.vector.tensor_copy(ctail[:, :, 64:128], xT[:, :, 576:640])
                return xT, ctail

            def emit_B(p, xT, fillers=(), stage_hooks=None):
                """qk matmuls for pair p -> qTp (DMA) and kTpk (drain).

                fillers (prev batch PV pieces) interleave after every other
                chain so the pair boundary has no PE gap."""
                sl = p % 2
                fi = 0
                hooks = stage_hooks or {}
                for ci_, fc in enumerate(FC_ORDER):
                    # alternate pools: doubles the psum recycle distance so
                    # chain drains have 4 chains of slack, not 2
                    if ci_ % 2 == 0:
                        tg = P["pg"].tile([128, 1024], F32, tag="pg")
                    else:
                        tg4 = P["ps"].tile([128, 2, 2, 256], F32, tag="ps")
                        tg = tg4.rearrange("p a b q -> p (a b q)")
                    for ks in range(KS):
                        nc.tensor.matmul(
                            tg[:, 0:512],
                            lhsT=wqkv16[:, ks, fc * 128 : (fc + 1) * 128],
                            rhs=xT[:, ks, 0:512],
                            start=(ks == 0),
                            stop=(ks == KS - 1),
                        )
                    for ks in range(KS):
                        nc.tensor.matmul(
                            tg[:, 512:640],
                            lhsT=wqkv16[:, ks, fc * 128 : (fc + 1) * 128],
                            rhs=xT[:, ks, 512:640],
                            start=(ks == 0),
                            stop=(ks == KS - 1),
                        )
                    if fc < KS:  # q features -> padded per-head tiles via DMA
                        qf = P["qkfc"].tile([128, PAIR_TOK], F16, tag="qkfc")
                        nc.vector.tensor_copy(qf[:], tg[:, 0:640])
                        nc.sync.dma_start(qTp[0:64, sl, 2 * fc, :], qf[0:64, :])
                        nc.sync.dma_start(qTp[64:128, sl, 2 * fc + 1, :], qf[64:128, :])
                    else:  # k features -> packed tile directly (alternate engines)
                        if fc % 2 == 0:
                            nc.vector.tensor_copy(kTpk[:, sl, fc - KS, :], tg[:, 0:640])
                        else:
                            nc.scalar.copy(kTpk[:, sl, fc - KS, :], tg[:, 0:640])
                    if fi < len(fillers):
                        fillers[fi]()
                        fi += 1
                    if ci_ in hooks:
                        hooks[ci_]()
                while fi < len(fillers):
                    fillers[fi]()
                    fi += 1

            def make_C_pair(p):
                """v matmuls for both batches of pair p; the two 64-token
                tails run packed as one 128-row pass (es chunk-2 pad rows
                are zero, so the cross-batch rows contribute nothing)."""
                psl = p % 2
                pieces = []

                def chunk_pieces(g, off):
                    sl = g % 2
                    btok = (g % 2) * N
                    holder = {}

                    def piece_a():
                        xT = xT_cur[g // 2]
                        tg = P["pg"].tile([128, 1024], F32, tag="pg")
                        holder["tg"] = tg
                        for ks in range(KS):
                            nc.tensor.matmul(
                                tg[:, 0:512],
                                lhsT=xT[:, ks, btok + off : btok + off + 128],
                                rhs=wqkv16[:, ks, 2 * C : 2 * C + 512],
                                start=(ks == 0),
                                stop=(ks == KS - 1),
                            )

                    def piece_b():
                        xT = xT_cur[g // 2]
                        tg = holder["tg"]
                        for ks in range(KS):
                            nc.tensor.matmul(
                                tg[:, 512:768],
                                lhsT=xT[:, ks, btok + off : btok + off + 128],
                                rhs=wqkv16[:, ks, 2 * C + 512 : 3 * C],
                                start=(ks == 0),
                                stop=(ks == KS - 1),
                            )
                        ci = off // 128
                        nc.vector.tensor_copy(
                            va[:, sl, ci, :, 0:64],
                            tg[:, 0:768].rearrange("p (h d) -> p h d", d=64),
                        )

                    return [piece_a, piece_b]

                def tail_pieces():
                    holder = {}

                    def piece_a():
                        ctail = ctail_cur[p]
                        tg = P["pg"].tile([128, 1024], F32, tag="pg")
                        holder["tg"] = tg
                        for ks in range(KS):
                            nc.tensor.matmul(
                                tg[:, 0:512],
                                lhsT=ctail[:, ks, :],
                                rhs=wqkv16[:, ks, 2 * C : 2 * C + 512],
                                start=(ks == 0),
                                stop=(ks == KS - 1),
                            )

                    def piece_b():
                        ctail = ctail_cur[p]
                        tg = holder["tg"]
                        for ks in range(KS):
                            nc.tensor.matmul(
                                tg[:, 512:768],
                                lhsT=ctail[:, ks, :],
                                rhs=wqkv16[:, ks, 2 * C + 512 : 3 * C],
                                start=(ks == 0),
                                stop=(ks == KS - 1),
                            )
                        # rows 0:64 = even batch tail, 64:128 = odd batch tail
                        nc.vector.tensor_copy(
                            va[0:64, 0, 2, :, 0:64],
                            tg[0:64, 0:768].rearrange("p (h d) -> p h d", d=64),
                        )
                        nc.vector.tensor_copy(
                            va[64:128, 1, 2, :, 0:64],
                            tg[64:128, 0:768].rearrange("p (h d) -> p h d", d=64),
                        )

                    return [piece_a, piece_b]

                g0, g1 = 2 * p, 2 * p + 1
                pieces += chunk_pieces(g0, 0) + chunk_pieces(g0, 128)
                pieces += chunk_pieces(g1, 0) + chunk_pieces(g1, 128)
                pieces += tail_pieces()
                return pieces

            KT_CHUNKS = [(0, 128), (128, 128), (256, 64)]

            def score_group(g, ci, hg):
                """One (key-chunk, head-group) score matmul pair + exp."""
                sl = g % 2
                psl = (g // 2) % 2
                btok = (g % 2) * N
                koff, ksz = KT_CHUNKS[ci]
                # odd batches' chunk-2 scores must land in partitions 64:128
                # (where their packed v-tail rows live): use a full-width
                # lhsT whose free cols 64:128 are the tail keys (cols 0:64
                # recompute chunk-1 keys, ignored)
                shift = ci == 2 and sl == 1
                r0 = 64 if shift else 0

                def run():
                    # two heads per matmul: both heads' padded q side by
                    # side (N=512); the packed kT chunk's parity halves
                    # each hit their own head, zeros kill cross terms
                    psc = P["ps"].tile([128, 2, 2, 256], F32, tag="ps")
                    for hp in range(2):
                        h0 = hg * 4 + hp * 2
                        k0 = btok + koff - (64 if shift else 0)
                        kw = 128 if shift else ksz
                        nc.tensor.matmul(
                            psc[0 : r0 + ksz, hp, :, :],
                            lhsT=kTpk[:, psl, h0 // 2, k0 : k0 + kw],
                            rhs=qTp[:, psl, h0 : h0 + 2, btok + 64 : btok + 320],
                            start=True,
                            stop=True,
                        )
                    nc.scalar.activation(
                        es[r0 : r0 + ksz, sl, ci, hg * 4 : hg * 4 + 4, :],
                        psc[r0 : r0 + ksz, :, :, :].rearrange("p a b q -> p (a b) q"),
                        mybir.ActivationFunctionType.Exp,
                        scale=0.125,
                    )

                return run

            ALL_GROUPS = [(ci, hg) for ci in range(3) for hg in range(3)]

            def emit_D(g, fillers, groups=None):
                """Scores + exp for batch g, interleaving filler pieces."""
                if groups is None:
                    groups = ALL_GROUPS
                fi = 0
                nf = len(fillers)
                ng = len(groups)
                for gi, (ci, hg) in enumerate(groups, 1):
                    score_group(g, ci, hg)()
                    want = (nf * gi) // ng
                    while fi < want:
                        fillers[fi]()
                        fi += 1
                while fi < len(fillers):
                    fillers[fi]()
                    fi += 1

            def make_E(g):
                """Template scores + exp for batch g (2 pieces)."""
                sl = g % 2
                psl = (g // 2) % 2
                btok = (g % 2) * N
                holder = {}

                def mm_piece():
                    tg = P["pg"].tile([128, 1024], F32, tag="pg")
                    holder["tg"] = tg
                    for hp in range(6):
                        h0 = 2 * hp
                        nc.tensor.matmul(
                            tg[0:64, h0 * 64 : (h0 + 2) * 64],
                            lhsT=kTpk[:, psl, hp, btok : btok + 64],
                            rhs=qTp[:, psl, h0 : h0 + 2, btok : btok + 64],
                            start=True,
                            stop=True,
                        )

                def exp_piece():
                    tg = holder["tg"]
                    nc.scalar.activation(
                        esm[0:64, sl, :, :],
                        tg[0:64, 0:768].rearrange("p (h q) -> p h q", q=64),
                        mybir.ActivationFunctionType.Exp,
                        scale=0.125,
                    )

                return [mm_piece, exp_piece]

            def _normalize(tg, qsz, qg, half, sl):
                # two DVE ops by necessity: the engine reads only one operand
                # from PSUM, so the reciprocal doubles as the PSUM->SBUF
                # staging step for the denominator
                po_v = tg[:qsz, 0:510].rearrange("p (h s) -> p h s", s=SLOT)
                rcp = P["rcp"].tile([128, 8], F32, tag="rcp")
                nc.vector.reciprocal(rcp[:qsz, 0:6], po_v[:, :, 64])
                nc.vector.tensor_tensor(
                    attn[:qsz, sl, qg, half * 384 : (half + 1) * 384].rearrange(
                        "p (h d) -> p h d", d=64
                    ),
                    po_v[:, :, 0:64],
                    rcp[:qsz, 0:6, None].to_broadcast([qsz, 6, 64]),
                    mybir.AluOpType.mult,
                )

            _alt = {"i": 0}

            def fgh_tile(alt):
                # post-D pieces can round-robin into the idle ps pool,
                # doubling the psum recycle distance in the final flush
                if alt and _alt["i"] % 2 == 1:
                    t4 = P["ps"].tile([128, 2, 2, 256], F32, tag="ps")
                    t = t4.rearrange("p a b q -> p (a b q)")
                else:
                    t = P["pg"].tile([128, 1024], F32, tag="pg")
                _alt["i"] += 1
                return t

            def make_FGH(g, ptail, alt_pv=False, alt_pj=False):
                """PV + normalize + attn^T + proj for batch g (deferred).

                Returns (pv_pieces, pj_pieces). attn^T via regular matmuls
                against the identity (weight-load overlaps); each at-piece
                drains to attnT cols 0:256 (DVE) and the 256:320 tail into
                the per-pair ptail staging tile (Act)."""
                sl = g % 2
                p = g // 2
                odd = g % 2 == 1

                def tpv(half):
                    def run():
                        tg = fgh_tile(alt_pv)
                        for j in range(6):
                            h = half * 6 + j
                            nc.tensor.matmul(
                                tg[0:64, j * SLOT : j * SLOT + 65],
                                lhsT=esm[:, sl, h, 0:64],
                                rhs=va[:, sl, 0, h, 0:65],
                                start=True,
                                stop=True,
                            )
                        _normalize(tg, 64, 0, half, sl)
                    return run

                def spv(qg, half):
                    def run():
                        tg = fgh_tile(alt_pv)
                        for j in range(6):
                            h = half * 6 + j
                            for ci in range(3):
                                nc.tensor.matmul(
                                    tg[0:128, j * SLOT : j * SLOT + 65],
                                    lhsT=es[:, sl, ci, h, (qg - 1) * 128 : qg * 128],
                                    rhs=va[:, sl, ci, h, 0:65],
                                    start=(ci == 0),
                                    stop=(ci == 2),
                                )
                        _normalize(tg, 128, qg, half, sl)
                    return run

                def at(fc):
                    def run():
                        tg = fgh_tile(alt_pv)
                        # overlap-packed: qg0 -> 0:128 (real 0:64),
                        # qg1 -> 64:192, qg2 -> 192:320
                        for qg, dst0 in ((0, 0), (1, 64), (2, 192)):
                            nc.tensor.matmul(
                                tg[:, dst0 : dst0 + 128],
                                lhsT=attn[0:128, sl, qg, fc * 128 : (fc + 1) * 128],
                                rhs=ident16[:, 0:128],
                                start=True,
                                stop=True,
                            )
                        nc.vector.tensor_copy(attnT[:, sl, fc, :], tg[:, 0:256])
                        if odd:
                            nc.scalar.copy(
                                ptail[:, fc, sl * 64 : sl * 64 + 64], tg[:, 256:320]
                            )
                        else:
                            nc.vector.tensor_copy(
                                ptail[:, fc, sl * 64 : sl * 64 + 64], tg[:, 256:320]
                            )
                    return run

                pv_pieces = (
                    [spv(1, 0), spv(2, 0), tpv(0), spv(1, 1)]
                    + [at(0), at(1), at(2)]
                    + [spv(2, 1), tpv(1), at(3), at(4), at(5)]
                )

                # proj + bias + out DMA
                row0 = g * N

                def pj(qoff):
                    def run():
                        tg = fgh_tile(alt_pj)
                        for ks in range(KS):
                            nc.tensor.matmul(
                                tg[:, 0:512],
                                lhsT=attnT[:, sl, ks, qoff : qoff + 128],
                                rhs=wproj16[:, ks, 0:512],
                                start=(ks == 0),
                                stop=(ks == KS - 1),
                            )
                        for ks in range(KS):
                            nc.tensor.matmul(
                                tg[:, 512:768],
                                lhsT=attnT[:, sl, ks, qoff : qoff + 128],
                                rhs=wproj16[:, ks, 512:768],
                                start=(ks == 0),
                                stop=(ks == KS - 1),
                            )
                        ost = P["outst"].tile([128, C], F16, tag="outst")
                        nc.vector.tensor_tensor(
                            ost[:, :], tg[:, 0:768], bias_bc[:, :],
                            mybir.AluOpType.add,
                        )
                        nc.sync.dma_start(
                            out_ap[row0 + qoff : row0 + qoff + 128, :], ost[:, :]
                        )
                    return run

                def pj_tail():
                    # both batches' 64-token tails in one 128-row pass
                    def run():
                        tg = fgh_tile(alt_pj)
                        for ks in range(KS):
                            nc.tensor.matmul(
                                tg[:, 0:512],
                                lhsT=ptail[:, ks, :],
                                rhs=wproj16[:, ks, 0:512],
                                start=(ks == 0),
                                stop=(ks == KS - 1),
                            )
                        for ks in range(KS):
                            nc.tensor.matmul(
                                tg[:, 512:768],
                                lhsT=ptail[:, ks, :],
                                rhs=wproj16[:, ks, 512:768],
                                start=(ks == 0),
                                stop=(ks == KS - 1),
                            )
                        ost = P["outst"].tile([128, C], F16, tag="outst")
                        nc.vector.tensor_tensor(
                            ost[:, :], tg[:, 0:768], bias_bc[:, :],
                            mybir.AluOpType.add,
                        )
                        g0row = 2 * p * N + 256
                        g1row = (2 * p + 1) * N + 256
                        nc.sync.dma_start(out_ap[g0row : g0row + 64, :], ost[0:64, :])
                        nc.sync.dma_start(out_ap[g1row : g1row + 64, :], ost[64:128, :])
                    return run

                pj_pieces = [pj(0), pj(128)]
                if odd:
                    pj_pieces.append(pj_tail())
                return pv_pieces, pj_pieces

            # ================= main schedule =================
            xT_cur, ctail_cur = {}, {}
            xT_cur[0], ctail_cur[0] = emit_xT(0)
            emit_warmup()
            emit_weight_load()
            emit_pads()

            stash_pv, stash_pj = [], []
            for p in range(NPAIR):
                last = p == NPAIR - 1
                if not last:
                    xT_cur[p + 1], ctail_cur[p + 1] = emit_xT(p + 1)
                emit_B(p, xT_cur[p], fillers=stash_pv)
                g0, g1 = 2 * p, 2 * p + 1
                ptail = P["ptail"].tile([128, KS, 128], F16, tag="ptail")
                Cp = make_C_pair(p)
                for pc in Cp[0:4]:
                    pc()
                emit_D(g0, stash_pj + Cp[4:] + make_E(g0))
                f_pv, f_pj = make_FGH(g0, ptail, alt_pj=last)
                if last:
                    # hold g6's proj back so its PE-heavy pieces can cover
                    # the final flush's normalize latencies; emit the last
                    # batch's score groups head-group-major so the flush's
                    # first PV pieces (heads 0-5, all chunks) unblock early
                    hgm = [(ci, hg) for hg in range(3) for ci in range(3)]
                    emit_D(g1, f_pv + make_E(g1), groups=hgm)
                    held_pj = f_pj
                else:
                    emit_D(g1, f_pv + make_E(g1) + f_pj)
                    held_pj = []
                stash_pv, stash_pj = make_FGH(
                    g1, ptail, alt_pv=last, alt_pj=last
                )
            pv = [stash_pv[i] for i in (0, 1, 2, 3, 7, 8)]
            ats = [stash_pv[i] for i in (4, 5, 6, 9, 10, 11)]
            final = pv + held_pj[0:1] + ats + held_pj[1:2] + stash_pj
            for pc in final:
                pc()

    nc.compile()
    return nc


@functools.cache
def _get_nc():
    return build_kernel()


def make_in_maps(x, wqkv, wproj, bias):
    x16 = x.reshape(B, N, C).astype(np.float16)
    wqkv16 = np.ascontiguousarray(wqkv.astype(np.float16))
    wproj16 = np.ascontiguousarray(wproj.astype(np.float16))
    bias = np.ascontiguousarray(bias.astype(np.float32))
    return [
        {
            "xT16": np.ascontiguousarray(
                x16[c * B_CORE : (c + 1) * B_CORE].reshape(TOK_CORE, C).T
            ),
            "W_qkv16": wqkv16,
            "W_proj16": wproj16,
            "b_proj": bias,
        }
        for c in range(NCORES)
    ]


def kernel(**inputs):
    x = np.ascontiguousarray(np.asarray(inputs["x"], dtype=np.float32))
    wqkv = np.ascontiguousarray(np.asarray(inputs["W_qkv"], dtype=np.float32))
    wproj = np.ascontiguousarray(np.asarray(inputs["W_proj"], dtype=np.float32))
    bias = np.ascontiguousarray(np.asarray(inputs["b_proj"], dtype=np.float32))
    t_h = int(inputs.get("t_h", 8))
    t_w = int(inputs.get("t_w", 8))
    assert t_h * t_w == 64, "kernel built for template length 64"
    assert x.shape == (B, N, C)

    nc = _get_nc()
    in_maps = make_in_maps(x, wqkv, wproj, bias)
    # the axon tunnel occasionally drops with a transient INTERNAL error at
    # result fetch; the kernel is deterministic, so retry a couple of times
    last_err = None
    for attempt in range(3):
        try:
            res = run_bass_kernel_spmd(nc, in_maps, core_ids=list(range(NCORES)))
            break
        except Exception as e:  # noqa: BLE001 - transient PJRT/tunnel errors
            last_err = e
            if attempt == 2:
                raise
            # observed device-wedge recovery takes tens of seconds
            time.sleep(10 + 30 * attempt)
    out = np.concatenate(
        [r["out"].astype(np.float32).reshape(B_CORE, N, C) for r in res.results],
        axis=0,
    )
    return out


if __name__ == "__main__":
    _get_nc()
    print("kernel_v4 built OK")


# revision 55
# speedup vs baseline: 1.0215x; 1.0215x over previous
"""Trainium2 Bass kernel for MixformerAttention (sparse attention) — v4.

Problem shape (hardcoded):
  x [B=64, N=320, C=768], W_qkv [768, 2304], W_proj [768, 768], b_proj [768]
  H=12 heads, Dh=64, template L=64, search=256. DP over batch on 8 cores.

v4 changes vs the 306us v2 baseline (trace-driven; 262.7-264.5us measured
across runs, +-1.5us run-to-run variance):
  * One-time pad memsets moved DVE->GpSimd: the DVE memsets were queued
    between the q/k psum drains and chopped the first pair (PE never
    sustained HAM warm-up; ran at 1.2 GHz until t=30us).
  * PE warm-up: 24 dummy matmuls on resident tiles run during the initial
    weight/x DMA window so HAM is at K=8/8 when real work starts; the PE
    now stays warm for the whole kernel (was: cold until 30us + 8
    mid-kernel re-throttles).
  * 64-row batch tails packed: the v-projection and output-projection
    tails of a pair's two batches run as one 128-row pass (PE matmul cost
    is column-count only, so half-empty row chunks wasted cycles).
    Staging: ctail (DVE copy from xT) for the v tail, ptail (drained from
    the attn^T psum) for the proj tail. Odd batches' chunk-2 scores are
    emitted into psum partitions 64:128 via a full-width kT lhsT whose
    cols 64:128 are the tail keys, lining up with the packed v-tail rows.
  * emit_B alternates its psum tiles between the pg and ps pools (ps is
    idle during B), doubling the drain slack from 2 to 4 chains — this was
    the single biggest win (~35us): chain k+2 no longer stalls on chain
    k's DVE drain whenever a PV-filler normalize sat between them.
  * Prev-batch PV/at pieces interleave into emit_B after every chain; the
    final flush runs all PVs first, weaves the held-back proj pieces of
    the second-to-last batch over the normalize latency, and round-robins
    final psum tiles across both pools (scores are done by then).
  * attn^T stays on the PE (identity matmuls). A DMA XBAR transpose
    variant was tried and reverted: each InstDmaTransposeAnt blocks its
    issuing engine queue ~1.2us, convoying either the SP prefetch/store
    triggers or the Act exp pipeline no matter where it was routed.
"""

import contextlib
import functools
import time

import numpy as np

import concourse.bacc as bacc
import concourse.mybir as mybir
from concourse.bass_utils import run_bass_kernel_spmd
from concourse.masks import make_identity
from concourse.tile import TileContext

F32 = mybir.dt.float32
F16 = mybir.dt.float16

NCORES = 8
B, N, C = 64, 320, 768
H, DH = 12, 64
KS = C // 128  # 6 contraction subtiles
B_CORE = B // NCORES  # 8
PAIR_TOK = 2 * N  # 640
NPAIR = B_CORE // 2  # 4
TOK_CORE = B_CORE * N  # 2560
SLOT = 85  # psum col stride per head in PV output (6 heads in 510 cols)

FC_ORDER = [0, 6, 1, 7, 2, 8, 3, 9, 4, 10, 5, 11]  # q/k feature chunk order


def build_kernel():
    nc = bacc.Bacc("TRN2", target_bir_lowering=False)
    x_t = nc.dram_tensor("xT16", [C, TOK_CORE], F16, kind="ExternalInput")
    wqkv_t = nc.dram_tensor("W_qkv16", [C, 3 * C], F16, kind="ExternalInput")
    wproj_t = nc.dram_tensor("W_proj16", [C, C], F16, kind="ExternalInput")
    bias_t = nc.dram_tensor("b_proj", [C], F32, kind="ExternalInput")
    out_t = nc.dram_tensor("out", [TOK_CORE, C], F16, kind="ExternalOutput")
    x_ap, out_ap = x_t.ap(), out_t.ap()

    with TileContext(nc) as tc:
        with contextlib.ExitStack() as ctx:
            P = {
                "const": ctx.enter_context(tc.tile_pool(name="const", bufs=1)),
                "stagep": ctx.enter_context(tc.tile_pool(name="stagep", bufs=1)),
                "xT": ctx.enter_context(tc.tile_pool(name="xT", bufs=2)),
                "qkfc": ctx.enter_context(tc.tile_pool(name="qkfc", bufs=3)),
                "outst": ctx.enter_context(tc.tile_pool(name="outst", bufs=2)),
                "rcp": ctx.enter_context(tc.tile_pool(name="rcp", bufs=4)),
                "ctail": ctx.enter_context(tc.tile_pool(name="ctail", bufs=2)),
                "ptail": ctx.enter_context(tc.tile_pool(name="ptail", bufs=2)),
                "pg": ctx.enter_context(tc.tile_pool(name="pg", bufs=2, space="PSUM")),
                "ps": ctx.enter_context(tc.tile_pool(name="ps", bufs=2, space="PSUM")),
            }
            const = P["const"]

            # ---- persistent constants ----
            wqkv16 = const.tile([128, KS, 3 * C], F16, tag="wqkv16")
            wproj16 = const.tile([128, KS, C], F16, tag="wproj16")
            bias_bc = const.tile([128, C], F32, tag="bias_bc")
            ident16 = const.tile([128, 128], F16, tag="ident16")
            warm16 = const.tile([128, 512], F16, tag="warm16")
            make_identity(nc, ident16)

            # ---- persistent double-slotted activation tiles ----
            # per-head padded q (slot = pair parity)
            qTp = const.tile([128, 2, H, PAIR_TOK], F16, tag="qTp")
            # packed kT feature chunks (2 heads per chunk)
            kTpk = const.tile([128, 2, KS, PAIR_TOK], F16, tag="kTpk")
            # v natural with ones column (slot = batch parity)
            va = const.tile([128, 2, 3, H, 66], F16, tag="va")
            # exp(scores) for search queries [key, h, q]  (256 = search q)
            es = const.tile([128, 2, 3, H, 256], F16, tag="es")
            # exp(scores) template [key<=64 padded, h, q0:64]
            esm = const.tile([128, 2, H, 64], F16, tag="esm")
            # attention rows (template 64 padded | search 128 | search 128)
            attn = const.tile([128, 2, 3, C], F16, tag="attn")
            # attn^T per batch [C-part, tok] (tail tokens 256:320 live in
            # the per-pair ptail staging tile instead)
            attnT = const.tile([128, 2, KS, 256], F16, tag="attnT")

            def emit_warmup():
                # keep PE busy through the HAM SHORT window during the
                # initial DMA wait so real matmuls start at 2.4 GHz
                nc.gpsimd.memset(warm16[:, :], 1.0)
                for _ in range(24):
                    tgw = P["ps"].tile([128, 1024], F32, tag="ps")
                    nc.tensor.matmul(
                        tgw[:, 0:512], lhsT=ident16[:, 0:128], rhs=warm16[:, :],
                        start=True, stop=True,
                    )

            def emit_weight_load():
                for ks in range(KS):
                    nc.sync.dma_start(
                        wqkv16[:, ks, :], wqkv_t.ap()[ks * 128 : (ks + 1) * 128, :]
                    )
                for ks in range(KS):
                    nc.sync.dma_start(
                        wproj16[:, ks, :], wproj_t.ap()[ks * 128 : (ks + 1) * 128, :]
                    )
                brow = P["stagep"].tile([128, C], F32, tag="stagep")
                nc.sync.dma_start(brow[0:1, 0:C], bias_t.ap().unsqueeze(0))
                nc.gpsimd.partition_broadcast(bias_bc[:, :], brow[0:1, 0:C])

            def emit_pads():
                # one-time pad zeroing, all on gpsimd (DVE memsets stalled
                # the early q/k psum drains in v2)
                nc.gpsimd.memset(qTp[64:128, :, 0:H:2, :], 0.0)  # even heads
                nc.gpsimd.memset(qTp[0:64, :, 1:H:2, :], 0.0)  # odd heads
                # key-chunk-2 padding: even batches' tail scores live in rows
                # 0:64 (pad 64:128), odd batches' in rows 64:128 (pad 0:64),
                # matching the packed v-tail row split
                nc.gpsimd.memset(es[64:128, 0, 2, :, :], 0.0)
                nc.gpsimd.memset(es[0:64, 1, 2, :, :], 0.0)
                nc.gpsimd.memset(esm[64:128, :, :, :], 0.0)  # template keys
                nc.gpsimd.memset(attn[64:128, :, 0, :], 0.0)  # template rows
                # unused half-rows of the packed v-tails (es rows are zero
                # there, but they must hold finite values for the PV matmul)
                nc.gpsimd.memset(va[64:128, 0, 2, :, :], 0.0)
                nc.gpsimd.memset(va[0:64, 1, 2, :, :], 0.0)
                nc.gpsimd.memset(va[:, :, :, :, 64], 1.0)  # ones column
                nc.gpsimd.memset(va[:, :, :, :, 65], 0.0)  # stride pad

            # ================= emission helpers =================

            def emit_xT(p):
                """DMA the pre-transposed x^T slab of pair p into SBUF.

                Both batches' 64-token tails are staged contiguously in a
                ctail tile (DVE copy from xT: a strided HBM DMA would be
                descriptor-rate-bound) so the packed v-tail matmul gets a
                2D lhsT."""
                xT = P["xT"].tile([128, KS, PAIR_TOK], F16, tag="xT")
                src = x_ap[:, p * PAIR_TOK : (p + 1) * PAIR_TOK]
                nc.sync.dma_start(xT[:], src.rearrange("(k p) t -> p k t", p=128))
                ctail = P["ctail"].tile([128, KS, 128], F16, tag="ctail")
                nc.vector.tensor_copy(ctail[:, :, 0:64], xT[:, :, 256:320])
                nc.vector.tensor_copy(ctail[:, :, 64:128], xT[:, :, 576:640])
                return xT, ctail

            def emit_B(p, xT, fillers=(), stage_hooks=None):
                """qk matmuls for pair p -> qTp (DMA) and kTpk (drain).

                fillers (prev batch PV pieces) interleave after every other
                chain so the pair boundary has no PE gap."""
                sl = p % 2
                fi = 0
                hooks = stage_hooks or {}
                for ci_, fc in enumerate(FC_ORDER):
                    # alternate pools: doubles the psum recycle distance so
                    # chain drains have 4 chains of slack, not 2
                    if ci_ % 2 == 0:
                        tg = P["pg"].tile([128, 1024], F32, tag="pg")
                    else:
                        tg4 = P["ps"].tile([128, 2, 2, 256], F32, tag="ps")
                        tg = tg4.rearrange("p a b q -> p (a b q)")
                    for ks in range(KS):
                        nc.tensor.matmul(
                            tg[:, 0:512],
                            lhsT=wqkv16[:, ks, fc * 128 : (fc + 1) * 128],
                            rhs=xT[:, ks, 0:512],
                            start=(ks == 0),
                            stop=(ks == KS - 1),
                        )
                    for ks in range(KS):
                        nc.tensor.matmul(
                            tg[:, 512:640],
                            lhsT=wqkv16[:, ks, fc * 128 : (fc + 1) * 128],
                            rhs=xT[:, ks, 512:640],
                            start=(ks == 0),
                            stop=(ks == KS - 1),
                        )
                    if fc < KS:  # q features -> padded per-head tiles via DMA
                        qf = P["qkfc"].tile([128, PAIR_TOK], F16, tag="qkfc")
                        nc.vector.tensor_copy(qf[:], tg[:, 0:640])
                        nc.sync.dma_start(qTp[0:64, sl, 2 * fc, :], qf[0:64, :])
                        nc.sync.dma_start(qTp[64:128, sl, 2 * fc + 1, :], qf[64:128, :])
                    else:  # k features -> packed tile directly (alternate engines)
                        if fc % 2 == 0:
                            nc.vector.tensor_copy(kTpk[:, sl, fc - KS, :], tg[:, 0:640])
                        else:
                            nc.scalar.copy(kTpk[:, sl, fc - KS, :], tg[:, 0:640])
                    if fi < len(fillers):
                        fillers[fi]()
                        fi += 1
                    if ci_ in hooks:
                        hooks[ci_]()
                while fi < len(fillers):
                    fillers[fi]()
                    fi += 1

            def make_C_pair(p):
                """v matmuls for both batches of pair p; the two 64-token
                tails run packed as one 128-row pass (es chunk-2 pad rows
                are zero, so the cross-batch rows contribute nothing)."""
                psl = p % 2
                pieces = []

                def chunk_pieces(g, off):
                    sl = g % 2
                    btok = (g % 2) * N
                    holder = {}

                    def piece_a():
                        xT = xT_cur[g // 2]
                        tg = P["pg"].tile([128, 1024], F32, tag="pg")
                        holder["tg"] = tg
                        for ks in range(KS):
                            nc.tensor.matmul(
                                tg[:, 0:512],
                                lhsT=xT[:, ks, btok + off : btok + off + 128],
                                rhs=wqkv16[:, ks, 2 * C : 2 * C + 512],
                                start=(ks == 0),
                                stop=(ks == KS - 1),
                            )

                    def piece_b():
                        xT = xT_cur[g // 2]
                        tg = holder["tg"]
                        for ks in range(KS):
                            nc.tensor.matmul(
                                tg[:, 512:768],
                                lhsT=xT[:, ks, btok + off : btok + off + 128],
                                rhs=wqkv16[:, ks, 2 * C + 512 : 3 * C],
                                start=(ks == 0),
                                stop=(ks == KS - 1),
                            )
                        ci = off // 128
                        nc.vector.tensor_copy(
                            va[:, sl, ci, :, 0:64],
                            tg[:, 0:768].rearrange("p (h d) -> p h d", d=64),
                        )

                    return [piece_a, piece_b]

                def tail_pieces():
                    holder = {}

                    def piece_a():
                        ctail = ctail_cur[p]
                        tg = P["pg"].tile([128, 1024], F32, tag="pg")
                        holder["tg"] = tg
                        for ks in range(KS):
                            nc.tensor.matmul(
                                tg[:, 0:512],
                                lhsT=ctail[:, ks, :],
                                rhs=wqkv16[:, ks, 2 * C : 2 * C + 512],
                                start=(ks == 0),
                                stop=(ks == KS - 1),
                            )

                    def piece_b():
                        ctail = ctail_cur[p]
                        tg = holder["tg"]
                        for ks in range(KS):
                            nc.tensor.matmul(
                                tg[:, 512:768],
                                lhsT=ctail[:, ks, :],
                                rhs=wqkv16[:, ks, 2 * C + 512 : 3 * C],
                                start=(ks == 0),
                                stop=(ks == KS - 1),
                            )
                        # rows 0:64 = even batch tail, 64:128 = odd batch tail
                        nc.vector.tensor_copy(
                            va[0:64, 0, 2, :, 0:64],
                            tg[0:64, 0:768].rearrange("p (h d) -> p h d", d=64),
                        )
                        nc.vector.tensor_copy(
                            va[64:128, 1, 2, :, 0:64],
                            tg[64:128, 0:768].rearrange("p (h d) -> p h d", d=64),
                        )

                    return [piece_a, piece_b]

                g0, g1 = 2 * p, 2 * p + 1
                pieces += chunk_pieces(g0, 0) + chunk_pieces(g0, 128)
                pieces += chunk_pieces(g1, 0) + chunk_pieces(g1, 128)
                pieces += tail_pieces()
                return pieces

            KT_CHUNKS = [(0, 128), (128, 128), (256, 64)]

            def score_group(g, ci, hg):
                """One (key-chunk, head-group) score matmul pair + exp."""
                sl = g % 2
                psl = (g // 2) % 2
                btok = (g % 2) * N
                koff, ksz = KT_CHUNKS[ci]
                # odd batches' chunk-2 scores must land in partitions 64:128
                # (where their packed v-tail rows live): use a full-width
                # lhsT whose free cols 64:128 are the tail keys (cols 0:64
                # recompute chunk-1 keys, ignored)
                shift = ci == 2 and sl == 1
                r0 = 64 if shift else 0

                def run():
                    # two heads per matmul: both heads' padded q side by
                    # side (N=512); the packed kT chunk's parity halves
                    # each hit their own head, zeros kill cross terms
                    psc = P["ps"].tile([128, 2, 2, 256], F32, tag="ps")
                    for hp in range(2):
                        h0 = hg * 4 + hp * 2
                        k0 = btok + koff - (64 if shift else 0)
                        kw = 128 if shift else ksz
                        nc.tensor.matmul(
                            psc[0 : r0 + ksz, hp, :, :],
                            lhsT=kTpk[:, psl, h0 // 2, k0 : k0 + kw],
                            rhs=qTp[:, psl, h0 : h0 + 2, btok + 64 : btok + 320],
                            start=True,
                            stop=True,
                        )
                    nc.scalar.activation(
                        es[r0 : r0 + ksz, sl, ci, hg * 4 : hg * 4 + 4, :],
                        psc[r0 : r0 + ksz, :, :, :].rearrange("p a b q -> p (a b) q"),
                        mybir.ActivationFunctionType.Exp,
                        scale=0.125,
                    )

                return run

            ALL_GROUPS = [(ci, hg) for ci in range(3) for hg in range(3)]

            def emit_D(g, fillers, groups=None):
                """Scores + exp for batch g, interleaving filler pieces."""
                if groups is None:
                    groups = ALL_GROUPS
                fi = 0
                nf = len(fillers)
                ng = len(groups)
                for gi, (ci, hg) in enumerate(groups, 1):
                    score_group(g, ci, hg)()
                    want = (nf * gi) // ng
                    while fi < want:
                        fillers[fi]()
                        fi += 1
                while fi < len(fillers):
                    fillers[fi]()
                    fi += 1

            def make_E(g):
                """Template scores + exp for batch g (2 pieces)."""
                sl = g % 2
                psl = (g // 2) % 2
                btok = (g % 2) * N
                holder = {}

                def mm_piece():
                    tg = P["pg"].tile([128, 1024], F32, tag="pg")
                    holder["tg"] = tg
                    for hp in range(6):
                        h0 = 2 * hp
                        nc.tensor.matmul(
                            tg[0:64, h0 * 64 : (h0 + 2) * 64],
                            lhsT=kTpk[:, psl, hp, btok : btok + 64],
                            rhs=qTp[:, psl, h0 : h0 + 2, btok : btok + 64],
                            start=True,
                            stop=True,
                        )

                def exp_piece():
                    tg = holder["tg"]
                    nc.scalar.activation(
                        esm[0:64, sl, :, :],
                        tg[0:64, 0:768].rearrange("p (h q) -> p h q", q=64),
                        mybir.ActivationFunctionType.Exp,
                        scale=0.125,
                    )

                return [mm_piece, exp_piece]

            def _normalize(tg, qsz, qg, half, sl):
                # two DVE ops by necessity: the engine reads only one operand
                # from PSUM, so the reciprocal doubles as the PSUM->SBUF
                # staging step for the denominator
                po_v = tg[:qsz, 0:510].rearrange("p (h s) -> p h s", s=SLOT)
                rcp = P["rcp"].tile([128, 8], F32, tag="rcp")
                nc.vector.reciprocal(rcp[:qsz, 0:6], po_v[:, :, 64])
                nc.vector.tensor_tensor(
                    attn[:qsz, sl, qg, half * 384 : (half + 1) * 384].rearrange(
                        "p (h d) -> p h d", d=64
                    ),
                    po_v[:, :, 0:64],
                    rcp[:qsz, 0:6, None].to_broadcast([qsz, 6, 64]),
                    mybir.AluOpType.mult,
                )

            _alt = {"i": 0}

            def fgh_tile(alt):
                # post-D pieces can round-robin into the idle ps pool,
                # doubling the psum recycle distance in the final flush
                if alt and _alt["i"] % 2 == 1:
                    t4 = P["ps"].tile([128, 2, 2, 256], F32, tag="ps")
                    t = t4.rearrange("p a b q -> p (a b q)")
                else:
                    t = P["pg"].tile([128, 1024], F32, tag="pg")
                _alt["i"] += 1
                return t

            def make_FGH(g, ptail, alt_pv=False, alt_pj=False):
                """PV + normalize + attn^T + proj for batch g (deferred).

                Returns (pv_pieces, pj_pieces). attn^T via regular matmuls
                against the identity (weight-load overlaps); each at-piece
                drains to attnT cols 0:256 (DVE) and the 256:320 tail into
                the per-pair ptail staging tile (Act)."""
                sl = g % 2
                p = g // 2
                odd = g % 2 == 1

                def tpv(half):
                    def run():
                        tg = fgh_tile(alt_pv)
                        for j in range(6):
                            h = half * 6 + j
                            nc.tensor.matmul(
                                tg[0:64, j * SLOT : j * SLOT + 65],
                                lhsT=esm[:, sl, h, 0:64],
                                rhs=va[:, sl, 0, h, 0:65],
                                start=True,
                                stop=True,
                            )
                        _normalize(tg, 64, 0, half, sl)
                    return run

                def spv(qg, half):
                    def run():
                        tg = fgh_tile(alt_pv)
                        for j in range(6):
                            h = half * 6 + j
                            for ci in range(3):
                                nc.tensor.matmul(
                                    tg[0:128, j * SLOT : j * SLOT + 65],
                                    lhsT=es[:, sl, ci, h, (qg - 1) * 128 : qg * 128],
                                    rhs=va[:, sl, ci, h, 0:65],
                                    start=(ci == 0),
                                    stop=(ci == 2),
                                )
                        _normalize(tg, 128, qg, half, sl)
                    return run

                def at(fc):
                    def run():
                        tg = fgh_tile(alt_pv)
                        # overlap-packed: qg0 -> 0:128 (real 0:64),
                        # qg1 -> 64:192, qg2 -> 192:320
                        for qg, dst0 in ((0, 0), (1, 64), (2, 192)):
                            nc.tensor.matmul(
                                tg[:, dst0 : dst0 + 128],
                                lhsT=attn[0:128, sl, qg, fc * 128 : (fc + 1) * 128],
                                rhs=ident16[:, 0:128],
                                start=True,
                                stop=True,
                            )
                        nc.vector.tensor_copy(attnT[:, sl, fc, :], tg[:, 0:256])
                        if odd:
                            nc.scalar.copy(
                                ptail[:, fc, sl * 64 : sl * 64 + 64], tg[:, 256:320]
                            )
                        else:
                            nc.vector.tensor_copy(
                                ptail[:, fc, sl * 64 : sl * 64 + 64], tg[:, 256:320]
                            )
                    return run

                pv_pieces = (
                    [spv(1, 0), spv(2, 0), tpv(0), at(0), at(1), at(2)]
                    + [spv(1, 1), spv(2, 1), tpv(1), at(3), at(4), at(5)]
                )

                # proj + bias + out DMA
                row0 = g * N

                def pj(qoff):
                    def run():
                        tg = fgh_tile(alt_pj)
                        for ks in range(KS):
                            nc.tensor.matmul(
                                tg[:, 0:512],
                                lhsT=attnT[:, sl, ks, qoff : qoff + 128],
                                rhs=wproj16[:, ks, 0:512],
                                start=(ks == 0),
                                stop=(ks == KS - 1),
                            )
                        for ks in range(KS):
                            nc.tensor.matmul(
                                tg[:, 512:768],
                                lhsT=attnT[:, sl, ks, qoff : qoff + 128],
                                rhs=wproj16[:, ks, 512:768],
                                start=(ks == 0),
                                stop=(ks == KS - 1),
                            )
                        ost = P["outst"].tile([128, C], F16, tag="outst")
                        nc.vector.tensor_tensor(
                            ost[:, :], tg[:, 0:768], bias_bc[:, :],
                            mybir.AluOpType.add,
                        )
                        nc.sync.dma_start(
                            out_ap[row0 + qoff : row0 + qoff + 128, :], ost[:, :]
                        )
                    return run

                def pj_tail():
                    # both batches' 64-token tails in one 128-row pass
                    def run():
                        tg = fgh_tile(alt_pj)
                        for ks in range(KS):
                            nc.tensor.matmul(
                                tg[:, 0:512],
                                lhsT=ptail[:, ks, :],
                                rhs=wproj16[:, ks, 0:512],
                                start=(ks == 0),
                                stop=(ks == KS - 1),
                            )
                        for ks in range(KS):
                            nc.tensor.matmul(
                                tg[:, 512:768],
                                lhsT=ptail[:, ks, :],
                                rhs=wproj16[:, ks, 512:768],
                                start=(ks == 0),
                                stop=(ks == KS - 1),
                            )
                        ost = P["outst"].tile([128, C], F16, tag="outst")
                        nc.vector.tensor_tensor(
                            ost[:, :], tg[:, 0:768], bias_bc[:, :],
                            mybir.AluOpType.add,
                        )
                        g0row = 2 * p * N + 256
                        g1row = (2 * p + 1) * N + 256
                        nc.sync.dma_start(out_ap[g0row : g0row + 64, :], ost[0:64, :])
                        nc.sync.dma_start(out_ap[g1row : g1row + 64, :], ost[64:128, :])
                    return run

                pj_pieces = [pj(0), pj(128)]
                if odd:
                    pj_pieces.append(pj_tail())
                return pv_pieces, pj_pieces

            # ================= main schedule =================
            xT_cur, ctail_cur = {}, {}
            xT_cur[0], ctail_cur[0] = emit_xT(0)
            emit_warmup()
            emit_weight_load()
            emit_pads()

            stash_pv, stash_pj = [], []
            for p in range(NPAIR):
                last = p == NPAIR - 1
                if not last:
                    xT_cur[p + 1], ctail_cur[p + 1] = emit_xT(p + 1)
                emit_B(p, xT_cur[p], fillers=stash_pv)
                g0, g1 = 2 * p, 2 * p + 1
                ptail = P["ptail"].tile([128, KS, 128], F16, tag="ptail")
                Cp = make_C_pair(p)
                for pc in Cp[0:4]:
                    pc()
                emit_D(g0, stash_pj + Cp[4:] + make_E(g0))
                f_pv, f_pj = make_FGH(g0, ptail, alt_pj=last)
                if last:
                    # hold g6's proj back so its PE-heavy pieces can cover
                    # the final flush's normalize latencies; emit the last
                    # batch's score groups head-group-major so the flush's
                    # first PV pieces (heads 0-5, all chunks) unblock early
                    hgm = [(ci, hg) for hg in range(3) for ci in range(3)]
                    emit_D(g1, f_pv + make_E(g1), groups=hgm)
                    held_pj = f_pj
                else:
                    emit_D(g1, f_pv + make_E(g1) + f_pj)
                    held_pj = []
                stash_pv, stash_pj = make_FGH(
                    g1, ptail, alt_pv=last, alt_pj=last
                )
            pv = [stash_pv[i] for i in (0, 1, 2, 6, 7, 8)]
            ats = [stash_pv[i] for i in (3, 4, 5, 9, 10, 11)]
            final = pv + held_pj[0:1] + ats + held_pj[1:2] + stash_pj
            for pc in final:
                pc()

    nc.compile()
    return nc


@functools.cache
def _get_nc():
    return build_kernel()


def make_in_maps(x, wqkv, wproj, bias):
    x16 = x.reshape(B, N, C).astype(np.float16)
    wqkv16 = np.ascontiguousarray(wqkv.astype(np.float16))
    wproj16 = np.ascontiguousarray(wproj.astype(np.float16))
    bias = np.ascontiguousarray(bias.astype(np.float32))
    return [
        {
            "xT16": np.ascontiguousarray(
                x16[c * B_CORE : (c + 1) * B_CORE].reshape(TOK_CORE, C).T
            ),
            "W_qkv16": wqkv16,
            "W_proj16": wproj16,
            "b_proj": bias,
        }
        for c in range(NCORES)
    ]


def kernel(**inputs):
    x = np.ascontiguousarray(np.asarray(inputs["x"], dtype=np.float32))
    wqkv = np.ascontiguousarray(np.asarray(inputs["W_qkv"], dtype=np.float32))
    wproj = np.ascontiguousarray(np.asarray(inputs["W_proj"], dtype=np.float32))
    bias = np.ascontiguousarray(np.asarray(inputs["b_proj"], dtype=np.float32))
    t_h = int(inputs.get("t_h", 8))
    t_w = int(inputs.get("t_w", 8))
    assert t_h * t_w == 64, "kernel built for template length 64"
    assert x.shape == (B, N, C)

    nc = _get_nc()
    in_maps = make_in_maps(x, wqkv, wproj, bias)
    # the axon tunnel occasionally drops with a transient INTERNAL error at
    # result fetch; the kernel is deterministic, so retry a couple of times
    last_err = None
    for attempt in range(3):
        try:
            res = run_bass_kernel_spmd(nc, in_maps, core_ids=list(range(NCORES)))
            break
        except Exception as e:  # noqa: BLE001 - transient PJRT/tunnel errors
            last_err = e
            if attempt == 2:
                raise
            # observed device-wedge recovery takes tens of seconds
            time.sleep(10 + 30 * attempt)
    out = np.concatenate(
        [r["out"].astype(np.float32).reshape(B_CORE, N, C) for r in res.results],
        axis=0,
    )
    return out


if __name__ == "__main__":
    _get_nc()
    print("kernel_v4 built OK")
